# revision 17
# baseline (speedup 1.0000x reference)
"""Bass/Tile TRN2 kernel for nn_ExpressionAttentionLayer.

Math per batch b (B=8, G=2048, D=64):
    K_fused = concat([K_gene, K_expr], -1) @ WK_w.T + WK_b      # (G, D)
    Q_fused = concat([Q_gene, Q_expr], -1) @ WQ_w.T + WQ_b      # (G, D)
    A       = softmax(Q_fused @ K_fused.T / sqrt(D), axis=-1)
    out     = (A * M) @ V_expr                                   # (G, D)

Sharding: data-parallel over batch; core i handles batch i (B == n_cores == 8).
No collectives.

The kernel computes the whole attention in TRANSPOSED space so that no
on-device transposes are needed at all.  The host wrapper supplies
layout/dtype-transformed inputs (all bf16):
  - QcatT/KcatT [2D, G]: concat([X_gene, X_expr], -1) transposed
  - WQT/WKT [2D, D]: projection weights transposed
  - MTr [2, G, G/2]: the gating mask transposed and pre-tiled so each
    [128, 1024] device tile is one contiguous 256KB block
  - Vr [128, G/128, D]: V_expr with the k-tile index moved inside
and receives outT [D, G] fp32, transposing it back on the host.

Per-core dataflow (all matmul inputs bf16, fp32 PSUM):
  qfT/kfT [128, G] = (WT|WT).T @ catT + bias: the projected Q/K transposed,
      duplicated on partitions 64-127 so logits can be row-tiled.
  for qh in {0,1} (1024 q columns each), kt in 0..15 (128 k rows each):
    logitsT psum[128,1024]: two K=64 matmuls row-tiled onto array rows
        0-63 / 64-127 -> run concurrently                        (PE)
    expT    [128,1024] bf16 = Exp(logitsT / 8)                   (ACT)
    emT     [128,1024] bf16 = expT * MT_tile                     (DVE)
    avden   psum[128,1024]: col-tiled pair per 512-col half:
        rows 0:64   += Vr[:,kt,:].T @ emT   (attention @ V)
        rows 64:128 += ones[128,64].T @ expT (softmax denominator,
        replicated across partitions)      -> run concurrently   (PE)
  finalize: recip = approx(1/denom) (DVE), outT = av * recip (DVE), DMA.

The kt-dependent PE work (avden) of iteration i is emitted between the
logits of i+1 and i+2 so the PE never waits on ACT/DVE of the same
iteration.
"""

from contextlib import ExitStack

import numpy as np
import ml_dtypes

import concourse.bass as bass
import concourse.tile as tile
from concourse import bacc, mybir
from concourse.bass_utils import run_bass_kernel_spmd

B, G, D = 8, 2048, 64
P = 128
NKT = G // P          # 16 k-tiles of 128 rows
NQH = 2               # q processed in 2 halves of 1024 columns
QW = G // NQH         # 1024
F32 = mybir.dt.float32
BF16 = mybir.dt.bfloat16
AF = mybir.ActivationFunctionType

N_CORES = 8
BF = ml_dtypes.bfloat16


def _emit(ctx: ExitStack, tc: tile.TileContext, io: dict):
    nc = tc.nc

    singles = ctx.enter_context(tc.tile_pool(name="singles", bufs=1))

    # PSUM pools: logits 2x2 banks + av/den 2x2 = 8 banks.
    ps_l = ctx.enter_context(tc.tile_pool(name="ps_l", bufs=2, space="PSUM"))
    ps_ad = ctx.enter_context(tc.tile_pool(name="ps_ad", bufs=2, space="PSUM"))

    # ---- HAM warmup: junk matmuls while the first DMAs land, so the PE
    # clock ramps toward 2.4 GHz before the projections.  They rotate
    # through the (otherwise idle during the prelude) ps_ad pool so their
    # WAW chain never blocks the projection matmuls on ps_l.
    junk = singles.tile([P, 512], BF16, tag="junk")
    nc.vector.memset(junk[:], 0.0)
    for _ in range(4):
        psw = ps_ad.tile([P, QW], F32, tag="ps_ad", name="ps_warm")
        nc.tensor.matmul(psw[:, :512], junk[:, 0:P], junk[:], start=True, stop=True)

    ones_bf = singles.tile([P, D], BF16, tag="ones")
    nc.gpsimd.memset(ones_bf[:], 1.0)

    # ---- small inputs: weights+biases host-packed (pre-transposed,
    # duplicated into both column halves so projections land replicated on
    # psum partitions 0-63 and 64-127) — one DMA each.  Big inputs are
    # issued from four different engine queues so the ~600ns per-DMA issue
    # cost does not serialize the prelude.
    wcmb = singles.tile([P, 4 * D], BF16, tag="wcmb")
    bcmb = singles.tile([P, 2], F32, tag="bcmb")
    nc.sync.dma_start(wcmb[:], io["Wcmb"][:, :])
    nc.sync.dma_start(bcmb[:], io["Bcmb"][:, :])
    wkT = wcmb[:, 0 : 2 * D]
    wqT = wcmb[:, 2 * D : 4 * D]
    wkb = bcmb[:, 0:1]
    wqb = bcmb[:, 1:2]

    kcat = singles.tile([2 * D, G], BF16, tag="kcat")
    qcat = singles.tile([2 * D, G], BF16, tag="qcat")
    v_bf = singles.tile([P, NKT, D], BF16, tag="v")
    nc.scalar.dma_start(kcat[:, 0:QW], io["KcatT"][:, 0:QW])
    nc.gpsimd.dma_start(qcat[:, 0:QW], io["QcatT"][:, 0:QW])
    nc.sync.dma_start(v_bf[:], io["Vr"][:, :, :])
    nc.scalar.dma_start(kcat[:, QW:G], io["KcatT"][:, QW:G])
    nc.gpsimd.dma_start(qcat[:, QW:G], io["QcatT"][:, QW:G])

    # ---- fused projections: fT[d, g] = WT.T @ catT + b  (bias on DVE),
    # emitted in the order the main loop consumes them.
    kfT = singles.tile([P, G], BF16, tag="kfT")
    qfT = singles.tile([P, G], BF16, tag="qfT")

    def emit_proj(c):
        # one 512-col slice of BOTH projections in one psum tile; the K half
        # copies out on ACT, the Q half on DVE, concurrently.
        ps = ps_l.tile([P, QW], F32, tag="ps_l", name="ps_proj")
        nc.tensor.matmul(ps[:, 0:512], wkT[:], kcat[:, c : c + 512], start=True, stop=True)
        nc.tensor.matmul(ps[:, 512:QW], wqT[:], qcat[:, c : c + 512], start=True, stop=True)
        nc.scalar.activation(kfT[:, c : c + 512], ps[:, 0:512], AF.Identity, bias=wkb)
        nc.vector.tensor_scalar_add(qfT[:, c : c + 512], ps[:, 512:QW], wqb)

    # slices 0:1024 gate the first logits (q half 0 + k tiles 0-7); the
    # 1024:2048 slices are injected into the first loop iterations (their
    # deadlines are kt=8 and qh=1).
    emit_proj(0)
    emit_proj(512)
    emit_proj(1024)
    emit_proj(1536)
    deferred_proj = []

    # ---- main attention loop over 32 (qh, kt) tiles ----
    mpool = ctx.enter_context(tc.tile_pool(name="mpool", bufs=10))
    epool = ctx.enter_context(tc.tile_pool(name="epool", bufs=3))
    empool = ctx.enter_context(tc.tile_pool(name="empool", bufs=3))
    opool = ctx.enter_context(tc.tile_pool(name="opool", bufs=4))
    rpool = ctx.enter_context(tc.tile_pool(name="rpool", bufs=4))

    mt_ap = io["MTr"]
    outT_ap = io["outT"]
    scale = float(1.0 / np.sqrt(np.float32(D)))

    mts = {}

    def issue_mt(g):
        if g < NQH * NKT:
            qh, kt = divmod(g, NKT)
            mt = mpool.tile([P, QW], BF16, tag="m", name="m")
            eng = nc.sync if g % 2 == 0 else nc.gpsimd
            eng.dma_start(mt[:], mt_ap[qh, kt * P : (kt + 1) * P, :])
            mts[g] = mt

    # no upfront MT prefetch: the prelude input DMAs get the full DMA
    # bandwidth; the window grows by 2 per iteration and catches up well
    # before the mul deadlines

    avden = [None, None]
    pending = []  # [(qh, kt, expT, emT), ...] — avden MMs run 2 iters late

    def emit_den_av(qh, kt, expT, emT):
        st, sp = kt == 0, kt == NKT - 1
        ad = avden[qh]
        for c in range(2):
            cs = slice(c * 512, (c + 1) * 512)
            nc.tensor.matmul(
                ad[0:D, cs], ones_bf[:], expT[:, cs], start=st, stop=sp
            )
            nc.tensor.matmul(
                ad[D : 2 * D, cs],
                v_bf[:, kt, :],
                emT[:, cs],
                start=st,
                stop=sp,
                tile_position=(0, 64),
            )

    def finalize_half(qh, c):
        # halves, spread over iterations so the DVE is not oversubscribed
        # in the iterations right after a qh finishes
        ad = avden[qh]
        cs = slice(c * 512, (c + 1) * 512)
        recip = rpool.tile([D, 512], F32, tag="recip", name="recip")
        nc.vector.reciprocal_approx_fast(recip[:], ad[0:D, cs])
        ob = opool.tile([D, 512], F32, tag="ob")
        nc.vector.tensor_mul(ob[:], ad[D : 2 * D, cs], recip[:])
        eng = nc.scalar if c == 0 else nc.sync
        eng.dma_start(outT_ap[:, qh * QW + c * 512 : qh * QW + (c + 1) * 512], ob[:])

    deferred_fin = []
    for g in range(NQH * NKT):
        qh, kt = divmod(g, NKT)
        while deferred_proj and deferred_proj[0][0] <= g:
            emit_proj(deferred_proj.pop(0)[1])
        while deferred_fin and deferred_fin[0][0] <= g:
            finalize_half(*deferred_fin.pop(0)[1])
        issue_mt(2 * g)
        issue_mt(2 * g + 1)
        mt = mts.pop(g)

        # Row-tiled logits: the two 512-col halves contract on array rows
        # 0-63 / 64-127 (operands live on those partition halves) and run
        # concurrently on the PE.
        psl = ps_l.tile([P, QW], F32, tag="ps_l")
        nc.tensor.matmul(
            psl[:, 0:512],
            kfT[0:D, kt * P : (kt + 1) * P],
            qfT[0:D, qh * QW : qh * QW + 512],
            start=True,
            stop=True,
        )
        nc.tensor.matmul(
            psl[:, 512:QW],
            kfT[D : 2 * D, kt * P : (kt + 1) * P],
            qfT[D : 2 * D, qh * QW + 512 : (qh + 1) * QW],
            start=True,
            stop=True,
        )

        # the avden matmuls of iteration g-2 run on PE here, so the ACT exp
        # and DVE mul of an iteration have two full periods before the PE
        # consumes their outputs (no per-iteration PE stall on that chain).
        if kt == 0:
            avden[qh] = ps_ad.tile([P, QW], F32, tag="ps_ad", name="ps_ad")
        if len(pending) == 2:
            pg = pending.pop(0)
            emit_den_av(*pg)
            if pg[1] == NKT - 1:
                finalize_half(pg[0], 0)
                finalize_half(pg[0], 1)

        expT = epool.tile([P, QW], BF16, tag="ex")
        nc.scalar.activation(expT[:], psl[:], AF.Exp, scale=scale)
        emT = empool.tile([P, QW], BF16, tag="em")
        nc.vector.tensor_mul(emT[:], expT[:], mt[:])
        pending.append((qh, kt, expT, emT))

    for pg in pending:
        emit_den_av(*pg)
        if pg[1] == NKT - 1:
            finalize_half(pg[0], 0)
            finalize_half(pg[0], 1)
    for _, args in deferred_fin:
        finalize_half(*args)


def _build():
    # Bacc (not plain Bass): its compile() legalizes sync waits
    # (move_matmul_waits_to_ldweights + generate_event_semaphores) which
    # walrus codegen requires (max 1 wait per instruction).
    nc = bacc.Bacc("TRN2", target_bir_lowering=False, debug=False)
    io = {}
    io["QcatT"] = nc.dram_tensor("QcatT", [2 * D, G], BF16, kind="ExternalInput").ap()
    io["KcatT"] = nc.dram_tensor("KcatT", [2 * D, G], BF16, kind="ExternalInput").ap()
    io["Vr"] = nc.dram_tensor("Vr", [P, NKT, D], BF16, kind="ExternalInput").ap()
    io["MTr"] = nc.dram_tensor("MTr", [NQH, G, QW], BF16, kind="ExternalInput").ap()
    io["Wcmb"] = nc.dram_tensor("Wcmb", [P, 4 * D], BF16, kind="ExternalInput").ap()
    io["Bcmb"] = nc.dram_tensor("Bcmb", [P, 2], F32, kind="ExternalInput").ap()
    io["outT"] = nc.dram_tensor("outT", [D, G], F32, kind="ExternalOutput").ap()

    with tile.TileContext(nc) as tc:
        with ExitStack() as ctx:
            _emit(ctx, tc, io)
    nc.compile()
    return nc


_NC = None


def _get_nc():
    global _NC
    if _NC is None:
        _NC = _build()
    return _NC


def kernel(**inputs) -> np.ndarray:
    return run_kernel_with_results(**inputs)[0]


def run_kernel_with_results(trace=False, **inputs):
    """Returns (full_output, BassKernelResults)."""
    nc = _get_nc()
    f32 = {k: np.asarray(v, dtype=np.float32) for k, v in inputs.items()}

    # Host-side layout/dtype prep (bf16, transposed operands).
    m_bf = f32["M"].astype(BF)                      # (B, G, G)
    # Wcmb cols: [WKT|WKT|WQT|WQT]; Bcmb cols: [WK_b, WQ_b] partition-duped.
    wkT = f32["WK_w"].T.astype(BF)                  # (2D, D)
    wqT = f32["WQ_w"].T.astype(BF)
    wcmb = np.concatenate([wkT, wkT, wqT, wqT], axis=1)    # (128, 4D)
    bcmb = np.stack(
        [np.tile(f32["WK_b"], 2), np.tile(f32["WQ_b"], 2)], axis=1
    ).astype(np.float32)                             # (128, 2)

    in_maps = []
    for c in range(N_CORES):
        kcatT = np.concatenate(
            [f32["K_gene"][c].T, f32["K_expr"][c].T], axis=0
        ).astype(BF)                                 # (2D, G)
        qcatT = np.concatenate(
            [f32["Q_gene"][c].T, f32["Q_expr"][c].T], axis=0
        ).astype(BF)
        vr = np.ascontiguousarray(
            f32["V_expr"][c].reshape(NKT, P, D).transpose(1, 0, 2)
        ).astype(BF)                                 # (P, NKT, D)
        # MT pre-tiled: MTr[qh, k, j] = M[c][qh*QW + j, k]
        mT = m_bf[c].T                               # (G, G) bf16 view
        mtr = np.ascontiguousarray(
            mT.reshape(G, NQH, QW).transpose(1, 0, 2)
        )                                            # (NQH, G, QW)
        in_maps.append(
            {
                "QcatT": qcatT,
                "KcatT": kcatT,
                "Vr": vr,
                "MTr": mtr,
                "Wcmb": wcmb,
                "Bcmb": bcmb,
            }
        )
    res = run_bass_kernel_spmd(nc, in_maps, list(range(N_CORES)), trace=trace)
    out = np.stack(
        [np.asarray(res.results[c]["outT"], dtype=np.float32).T for c in range(N_CORES)],
        axis=0,
    )
    return np.ascontiguousarray(out), res


# revision 18
# speedup vs baseline: 1.0242x; 1.0242x over previous
"""Bass/Tile TRN2 kernel for nn_ExpressionAttentionLayer.

Math per batch b (B=8, G=2048, D=64):
    K_fused = concat([K_gene, K_expr], -1) @ WK_w.T + WK_b      # (G, D)
    Q_fused = concat([Q_gene, Q_expr], -1) @ WQ_w.T + WQ_b      # (G, D)
    A       = softmax(Q_fused @ K_fused.T / sqrt(D), axis=-1)
    out     = (A * M) @ V_expr                                   # (G, D)

Sharding: data-parallel over batch; core i handles batch i (B == n_cores == 8).
No collectives.

The kernel computes the whole attention in TRANSPOSED space so that no
on-device transposes are needed at all.  The host wrapper supplies
layout/dtype-transformed inputs (all bf16):
  - QcatT/KcatT [2D, G]: concat([X_gene, X_expr], -1) transposed
  - WQT/WKT [2D, D]: projection weights transposed
  - MTr [2, G, G/2]: the gating mask transposed and pre-tiled so each
    [128, 1024] device tile is one contiguous 256KB block
  - Vr [128, G/128, D]: V_expr with the k-tile index moved inside
and receives outT [D, G] fp32, transposing it back on the host.

Per-core dataflow (all matmul inputs bf16, fp32 PSUM):
  qfT/kfT [128, G] = (WT|WT).T @ catT + bias: the projected Q/K transposed,
      duplicated on partitions 64-127 so logits can be row-tiled.
  for qh in {0,1} (1024 q columns each), kt in 0..15 (128 k rows each):
    logitsT psum[128,1024]: two K=64 matmuls row-tiled onto array rows
        0-63 / 64-127 -> run concurrently                        (PE)
    expT    [128,1024] bf16 = Exp(logitsT / 8)                   (ACT)
    emT     [128,1024] bf16 = expT * MT_tile                     (DVE)
    avden   psum[128,1024]: col-tiled pair per 512-col half:
        rows 0:64   += Vr[:,kt,:].T @ emT   (attention @ V)
        rows 64:128 += ones[128,64].T @ expT (softmax denominator,
        replicated across partitions)      -> run concurrently   (PE)
  finalize: recip = approx(1/denom) (DVE), outT = av * recip (DVE), DMA.

The kt-dependent PE work (avden) of iteration i is emitted between the
logits of i+1 and i+2 so the PE never waits on ACT/DVE of the same
iteration.
"""

from contextlib import ExitStack

import numpy as np
import ml_dtypes

import concourse.bass as bass
import concourse.tile as tile
from concourse import bacc, mybir
from concourse.bass_utils import run_bass_kernel_spmd

B, G, D = 8, 2048, 64
P = 128
NKT = G // P          # 16 k-tiles of 128 rows
NQH = 2               # q processed in 2 halves of 1024 columns
QW = G // NQH         # 1024
F32 = mybir.dt.float32
BF16 = mybir.dt.bfloat16
AF = mybir.ActivationFunctionType

N_CORES = 8
BF = ml_dtypes.bfloat16


def _emit(ctx: ExitStack, tc: tile.TileContext, io: dict):
    nc = tc.nc

    singles = ctx.enter_context(tc.tile_pool(name="singles", bufs=1))

    # PSUM pools: logits 2x2 banks + av/den 2x2 = 8 banks.
    ps_l = ctx.enter_context(tc.tile_pool(name="ps_l", bufs=2, space="PSUM"))
    ps_ad = ctx.enter_context(tc.tile_pool(name="ps_ad", bufs=2, space="PSUM"))

    # ---- HAM warmup: junk matmuls while the first DMAs land, so the PE
    # clock ramps toward 2.4 GHz before the projections.  They rotate
    # through the (otherwise idle during the prelude) ps_ad pool so their
    # WAW chain never blocks the projection matmuls on ps_l.
    junk = singles.tile([P, 512], BF16, tag="junk")
    nc.vector.memset(junk[:], 0.0)
    for _ in range(3):
        psw = ps_ad.tile([P, QW], F32, tag="ps_ad", name="ps_warm")
        nc.tensor.matmul(psw[:, :512], junk[:, 0:P], junk[:], start=True, stop=True)

    ones_bf = singles.tile([P, D], BF16, tag="ones")
    nc.gpsimd.memset(ones_bf[:], 1.0)

    # ---- small inputs: weights+biases host-packed (pre-transposed,
    # duplicated into both column halves so projections land replicated on
    # psum partitions 0-63 and 64-127) — one DMA each.  Big inputs are
    # issued from four different engine queues so the ~600ns per-DMA issue
    # cost does not serialize the prelude.
    wcmb = singles.tile([P, 4 * D], BF16, tag="wcmb")
    bcmb = singles.tile([P, 2], F32, tag="bcmb")
    nc.sync.dma_start(wcmb[:], io["Wcmb"][:, :])
    nc.sync.dma_start(bcmb[:], io["Bcmb"][:, :])
    wkT = wcmb[:, 0 : 2 * D]
    wqT = wcmb[:, 2 * D : 4 * D]
    wkb = bcmb[:, 0:1]
    wqb = bcmb[:, 1:2]

    kcat = singles.tile([2 * D, G], BF16, tag="kcat")
    qcat = singles.tile([2 * D, G], BF16, tag="qcat")
    v_bf = singles.tile([P, NKT, D], BF16, tag="v")
    nc.scalar.dma_start(kcat[:, 0:QW], io["KcatT"][:, 0:QW])
    nc.gpsimd.dma_start(qcat[:, 0:QW], io["QcatT"][:, 0:QW])
    nc.sync.dma_start(v_bf[:], io["Vr"][:, :, :])
    nc.scalar.dma_start(kcat[:, QW:G], io["KcatT"][:, QW:G])
    nc.gpsimd.dma_start(qcat[:, QW:G], io["QcatT"][:, QW:G])

    # ---- fused projections: fT[d, g] = WT.T @ catT + b  (bias on DVE),
    # emitted in the order the main loop consumes them.
    kfT = singles.tile([P, G], BF16, tag="kfT")
    qfT = singles.tile([P, G], BF16, tag="qfT")

    def emit_proj(c):
        # one 512-col slice of BOTH projections in one psum tile; the K half
        # copies out on ACT, the Q half on DVE, concurrently.
        ps = ps_l.tile([P, QW], F32, tag="ps_l", name="ps_proj")
        nc.tensor.matmul(ps[:, 0:512], wkT[:], kcat[:, c : c + 512], start=True, stop=True)
        nc.tensor.matmul(ps[:, 512:QW], wqT[:], qcat[:, c : c + 512], start=True, stop=True)
        nc.scalar.activation(kfT[:, c : c + 512], ps[:, 0:512], AF.Identity, bias=wkb)
        nc.vector.tensor_scalar_add(qfT[:, c : c + 512], ps[:, 512:QW], wqb)

    # slices 0:1024 gate the first logits (q half 0 + k tiles 0-7); the
    # 1024:2048 slices are injected into the first loop iterations (their
    # deadlines are kt=8 and qh=1).
    emit_proj(0)
    emit_proj(512)
    emit_proj(1024)
    emit_proj(1536)
    deferred_proj = []

    # ---- main attention loop over 32 (qh, kt) tiles ----
    mpool = ctx.enter_context(tc.tile_pool(name="mpool", bufs=10))
    epool = ctx.enter_context(tc.tile_pool(name="epool", bufs=3))
    empool = ctx.enter_context(tc.tile_pool(name="empool", bufs=3))
    opool = ctx.enter_context(tc.tile_pool(name="opool", bufs=4))
    rpool = ctx.enter_context(tc.tile_pool(name="rpool", bufs=4))

    mt_ap = io["MTr"]
    outT_ap = io["outT"]
    scale = float(1.0 / np.sqrt(np.float32(D)))

    mts = {}

    def issue_mt(g):
        if g < NQH * NKT and g not in mts:
            qh, kt = divmod(g, NKT)
            mt = mpool.tile([P, QW], BF16, tag="m", name="m")
            # first tiles ride the scalar DMA queue BEHIND the kcat input
            # transfers (hardware processes a queue in order), so the MT
            # stream cannot starve the prelude inputs; the rest alternate
            # sync/gpsimd
            if g < 6:
                eng = nc.scalar
            else:
                eng = nc.sync if g % 2 == 0 else nc.gpsimd
            eng.dma_start(mt[:], mt_ap[qh, kt * P : (kt + 1) * P, :])
            mts[g] = mt

    for g in range(6):
        issue_mt(g)

    avden = [None, None]
    pending = []  # [(qh, kt, expT, emT), ...] — avden MMs run 2 iters late

    def emit_den_av(qh, kt, expT, emT):
        st, sp = kt == 0, kt == NKT - 1
        ad = avden[qh]
        for c in range(2):
            cs = slice(c * 512, (c + 1) * 512)
            nc.tensor.matmul(
                ad[0:D, cs], ones_bf[:], expT[:, cs], start=st, stop=sp
            )
            nc.tensor.matmul(
                ad[D : 2 * D, cs],
                v_bf[:, kt, :],
                emT[:, cs],
                start=st,
                stop=sp,
                tile_position=(0, 64),
            )

    def finalize_half(qh, c):
        # halves, spread over iterations so the DVE is not oversubscribed
        # in the iterations right after a qh finishes
        ad = avden[qh]
        cs = slice(c * 512, (c + 1) * 512)
        recip = rpool.tile([D, 512], F32, tag="recip", name="recip")
        nc.vector.reciprocal_approx_fast(recip[:], ad[0:D, cs])
        ob = opool.tile([D, 512], F32, tag="ob")
        nc.vector.tensor_mul(ob[:], ad[D : 2 * D, cs], recip[:])
        eng = nc.scalar if c == 0 else nc.sync
        eng.dma_start(outT_ap[:, qh * QW + c * 512 : qh * QW + (c + 1) * 512], ob[:])

    deferred_fin = []
    for g in range(NQH * NKT):
        qh, kt = divmod(g, NKT)
        while deferred_proj and deferred_proj[0][0] <= g:
            emit_proj(deferred_proj.pop(0)[1])
        while deferred_fin and deferred_fin[0][0] <= g:
            finalize_half(*deferred_fin.pop(0)[1])
        issue_mt(2 * g + 6)
        issue_mt(2 * g + 7)
        mt = mts.pop(g)

        # Row-tiled logits: the two 512-col halves contract on array rows
        # 0-63 / 64-127 (operands live on those partition halves) and run
        # concurrently on the PE.
        psl = ps_l.tile([P, QW], F32, tag="ps_l")
        nc.tensor.matmul(
            psl[:, 0:512],
            kfT[0:D, kt * P : (kt + 1) * P],
            qfT[0:D, qh * QW : qh * QW + 512],
            start=True,
            stop=True,
        )
        nc.tensor.matmul(
            psl[:, 512:QW],
            kfT[D : 2 * D, kt * P : (kt + 1) * P],
            qfT[D : 2 * D, qh * QW + 512 : (qh + 1) * QW],
            start=True,
            stop=True,
        )

        # the avden matmuls of iteration g-2 run on PE here, so the ACT exp
        # and DVE mul of an iteration have two full periods before the PE
        # consumes their outputs (no per-iteration PE stall on that chain).
        if kt == 0:
            avden[qh] = ps_ad.tile([P, QW], F32, tag="ps_ad", name="ps_ad")
        if len(pending) == 2:
            pg = pending.pop(0)
            emit_den_av(*pg)
            if pg[1] == NKT - 1:
                finalize_half(pg[0], 0)
                finalize_half(pg[0], 1)

        expT = epool.tile([P, QW], BF16, tag="ex")
        nc.scalar.activation(expT[:], psl[:], AF.Exp, scale=scale)
        emT = empool.tile([P, QW], BF16, tag="em")
        nc.vector.tensor_mul(emT[:], expT[:], mt[:])
        pending.append((qh, kt, expT, emT))

    for pg in pending:
        emit_den_av(*pg)
        if pg[1] == NKT - 1:
            finalize_half(pg[0], 0)
            finalize_half(pg[0], 1)
    for _, args in deferred_fin:
        finalize_half(*args)


def _build():
    # Bacc (not plain Bass): its compile() legalizes sync waits
    # (move_matmul_waits_to_ldweights + generate_event_semaphores) which
    # walrus codegen requires (max 1 wait per instruction).
    nc = bacc.Bacc("TRN2", target_bir_lowering=False, debug=False)
    io = {}
    io["QcatT"] = nc.dram_tensor("QcatT", [2 * D, G], BF16, kind="ExternalInput").ap()
    io["KcatT"] = nc.dram_tensor("KcatT", [2 * D, G], BF16, kind="ExternalInput").ap()
    io["Vr"] = nc.dram_tensor("Vr", [P, NKT, D], BF16, kind="ExternalInput").ap()
    io["MTr"] = nc.dram_tensor("MTr", [NQH, G, QW], BF16, kind="ExternalInput").ap()
    io["Wcmb"] = nc.dram_tensor("Wcmb", [P, 4 * D], BF16, kind="ExternalInput").ap()
    io["Bcmb"] = nc.dram_tensor("Bcmb", [P, 2], F32, kind="ExternalInput").ap()
    io["outT"] = nc.dram_tensor("outT", [D, G], F32, kind="ExternalOutput").ap()

    with tile.TileContext(nc) as tc:
        with ExitStack() as ctx:
            _emit(ctx, tc, io)
    nc.compile()
    return nc


_NC = None


def _get_nc():
    global _NC
    if _NC is None:
        _NC = _build()
    return _NC


def kernel(**inputs) -> np.ndarray:
    return run_kernel_with_results(**inputs)[0]


def run_kernel_with_results(trace=False, **inputs):
    """Returns (full_output, BassKernelResults)."""
    nc = _get_nc()
    f32 = {k: np.asarray(v, dtype=np.float32) for k, v in inputs.items()}

    # Host-side layout/dtype prep (bf16, transposed operands).
    m_bf = f32["M"].astype(BF)                      # (B, G, G)
    # Wcmb cols: [WKT|WKT|WQT|WQT]; Bcmb cols: [WK_b, WQ_b] partition-duped.
    wkT = f32["WK_w"].T.astype(BF)                  # (2D, D)
    wqT = f32["WQ_w"].T.astype(BF)
    wcmb = np.concatenate([wkT, wkT, wqT, wqT], axis=1)    # (128, 4D)
    bcmb = np.stack(
        [np.tile(f32["WK_b"], 2), np.tile(f32["WQ_b"], 2)], axis=1
    ).astype(np.float32)                             # (128, 2)

    in_maps = []
    for c in range(N_CORES):
        kcatT = np.concatenate(
            [f32["K_gene"][c].T, f32["K_expr"][c].T], axis=0
        ).astype(BF)                                 # (2D, G)
        qcatT = np.concatenate(
            [f32["Q_gene"][c].T, f32["Q_expr"][c].T], axis=0
        ).astype(BF)
        vr = np.ascontiguousarray(
            f32["V_expr"][c].reshape(NKT, P, D).transpose(1, 0, 2)
        ).astype(BF)                                 # (P, NKT, D)
        # MT pre-tiled: MTr[qh, k, j] = M[c][qh*QW + j, k]
        mT = m_bf[c].T                               # (G, G) bf16 view
        mtr = np.ascontiguousarray(
            mT.reshape(G, NQH, QW).transpose(1, 0, 2)
        )                                            # (NQH, G, QW)
        in_maps.append(
            {
                "QcatT": qcatT,
                "KcatT": kcatT,
                "Vr": vr,
                "MTr": mtr,
                "Wcmb": wcmb,
                "Bcmb": bcmb,
            }
        )
    res = run_bass_kernel_spmd(nc, in_maps, list(range(N_CORES)), trace=trace)
    out = np.stack(
        [np.asarray(res.results[c]["outT"], dtype=np.float32).T for c in range(N_CORES)],
        axis=0,
    )
    return np.ascontiguousarray(out), res
